# revision 16
# baseline (speedup 1.0000x reference)
"""TRN2 Bass kernel for nn_CustomAttnProcessor (B=8, S=1024, C=1280, H=20).

Strategy (v3 — interleaved software pipeline, all-bf16 inputs):
  - Batch-parallel: one batch element per NeuronCore (8 cores), no collectives.
  - Host pre-transposes inputs (xt = X[b].T, w*t = W.T) and casts to bf16;
    kernel emits Y^T in fp32; host transposes back.
  - v1 ran the whole attention phase with the PE HAM clock-gate stuck at
    K=4/8 (1.2 GHz): scores->exp->PV left too many PE idle gaps.  v2/v3
    interleave the Q/K/V projections of c-tile ct+1 with the attention of
    heads 2ct/2ct+1 at matmul granularity, keeping the PE densely busy
    (warm, 2.4 GHz) and hiding ~180us of serial ScalarE exp behind tensor
    work.  The O-projection runs as a dense tail, with its first chain
    pulled into the last attention round as filler (at-tiles 0..8 only —
    tile 9 is produced by that round, and Tile deps are emission-ordered).
  - Head-pair scores are emitted back-to-back so bass's auto tile_position
    ((0,0) for even heads, (64,0) for odd heads, tile_size (64,128)) lets
    the PE overlap the two 64-contraction matmuls (2x row tiling).
  - q is processed in halves of 512 so the PV accumulators for a head pair
    fit in 2 PSUM banks ([65, 512+512]); scores pair tiles are [128, 1024]
    (2 banks, one exp ACTIVATE of N=1024 covers both heads).
  - All matmul operands bf16 (inputs rounded on host; rel err ~5e-3 vs the
    2e-2 gate), accumulation fp32 in PSUM.  bf16 halves HBM traffic and
    SBUF footprint vs fp32r and enables FWL weight loads.
  - PV uses the ones-augmented-V trick: stationary [128, 65] whose 65th
    column accumulates the softmax denominator for free.
"""

import sys
import types

import numpy as np

B, S, C, H = 8, 1024, 1280, 20
D = C // H              # 64 head dim
P = 128
NCT = C // P            # 10 c-tiles
NST = S // P            # 8 s-tiles
SCALE = float(D) ** -0.5

# V-projection chunks: (col_offset, width) and the rounds their m-chains run
V_CHUNKS = [(0, 256), (256, 512), (768, 512)]

_cache = {}

last_exec_time_ns = None


def _install_profile_hook():
    """antenv.axon_hooks is absent in this container; reconstruct it from
    trn_agent_boot so BASS_TRACE=1 profiling works."""
    if "antenv.axon_hooks" in sys.modules:
        return
    mod = types.ModuleType("antenv.axon_hooks")
    state = {"hook": None}
    mod.set_axon_ntff_profile_hook = lambda h: state.update(hook=h)
    mod.get_axon_ntff_profile_hook = lambda: state["hook"]
    sys.modules["antenv.axon_hooks"] = mod
    try:
        from trn_agent_boot.trn_boot import _ntff_profile_via_ctypes

        hook = _ntff_profile_via_ctypes("/opt/axon/libaxon_pjrt.so")
        if hook is not None:
            mod.set_axon_ntff_profile_hook(hook)
    except Exception:
        pass


def _build():
    import concourse.mybir as mybir
    from concourse import bacc
    from concourse.tile import TileContext

    f32 = mybir.dt.float32
    bf16 = mybir.dt.bfloat16
    EXP = mybir.ActivationFunctionType.Exp

    nc = bacc.Bacc()
    xt = nc.declare_dram_parameter("xt", [C, S], bf16, isOutput=False)
    wq = nc.declare_dram_parameter("wq", [C, C], bf16, isOutput=False)
    wk = nc.declare_dram_parameter("wk", [C, C], bf16, isOutput=False)
    wv = nc.declare_dram_parameter("wv", [C, C], bf16, isOutput=False)
    wo = nc.declare_dram_parameter("wo", [C, C], bf16, isOutput=False)
    bo = nc.declare_dram_parameter("bo", [P, NCT], f32, isOutput=False)
    yt = nc.declare_dram_parameter("yt", [C, S], f32, isOutput=True)

    with TileContext(nc) as tc:
        with (
            tc.tile_pool(name="big", bufs=1) as big,
            tc.tile_pool(name="wqk", bufs=2) as wqkp,
            tc.tile_pool(name="wvp", bufs=2) as wvp,
            tc.tile_pool(name="wop", bufs=6) as wop,
            tc.tile_pool(name="ptp", bufs=10) as ptp,
            tc.tile_pool(name="rbp", bufs=4) as rbp,
            tc.tile_pool(name="rcpp", bufs=2) as rcpp,
            tc.tile_pool(name="ytp", bufs=2) as ytp,
            tc.tile_pool(name="smt", bufs=1) as smt,
            tc.tile_pool(name="psp", bufs=2, space="PSUM") as psp,
            tc.tile_pool(name="pop", bufs=1, space="PSUM") as pop,
            tc.tile_pool(name="pjp", bufs=2, space="PSUM") as pjp,
        ):
            # ---- persistent tiles ----
            xt_sb = big.tile([P, NCT, S], bf16, tag="xt")
            qt_sb = big.tile([P, NCT, S], bf16, tag="qt")
            kt_sb = big.tile([P, NCT, S], bf16, tag="kt")
            v_sb = big.tile([P, NST, H * 65], bf16, tag="v")
            at_sb = big.tile([P, NCT, S], bf16, tag="at")
            bo_sb = smt.tile([P, NCT], f32, tag="bo")
            ones_sb = smt.tile([P, H], bf16, tag="ones")

            xt_d = xt.rearrange("(t p) s -> t p s", p=P)
            yt_d = yt.rearrange("(t p) s -> t p s", p=P)

            # ---- weight staging ----
            wqk_tiles = {}

            def load_wqk(ct):
                t = wqkp.tile([P, NCT, 2 * P], bf16, tag="wqk", name="wqk_t")
                nc.sync.dma_start(
                    out=t[:, :, 0:P],
                    in_=wq[:, ct * P:(ct + 1) * P].rearrange("(t p) o -> p t o", p=P),
                )
                nc.sync.dma_start(
                    out=t[:, :, P:2 * P],
                    in_=wk[:, ct * P:(ct + 1) * P].rearrange("(t p) o -> p t o", p=P),
                )
                wqk_tiles[ct] = t

            wv_tiles = {}

            def load_wv(ch):
                off, width = V_CHUNKS[ch]
                t = wvp.tile([P, NCT, width], bf16, tag="wv", name="wv_t")
                nc.sync.dma_start(
                    out=t,
                    in_=wv[:, off:off + width].rearrange("(t p) o -> p t o", p=P),
                )
                wv_tiles[ch] = t

            wo_tiles = {}

            def load_wo(i):
                t = wop.tile([P, NCT, P], bf16, tag="wo", name="wo_t")
                nc.scalar.dma_start(
                    out=t,
                    in_=wo[:, i * P:(i + 1) * P].rearrange("(t p) o -> p t o", p=P),
                )
                wo_tiles[i] = t

            # ---- prologue DMAs, in consumption order, two queues ----
            # Per-DMA queue-issue overhead is ~0.9us, so split across the
            # sync and scalar (Activation) DGE queues and order by first use:
            # xt0/xt1, then the Q/K weight halves, then remaining xt chunks.
            t0 = wqkp.tile([P, NCT, 2 * P], bf16, tag="wqk", name="wqk_t")
            wqk_tiles[0] = t0
            wq0 = wq[:, 0:P].rearrange("(t p) o -> p t o", p=P)
            wk0 = wk[:, 0:P].rearrange("(t p) o -> p t o", p=P)
            nc.sync.dma_start(out=xt_sb[:, 0, :], in_=xt_d[0])
            nc.scalar.dma_start(out=xt_sb[:, 1, :], in_=xt_d[1])
            nc.sync.dma_start(out=t0[:, :, 0:P], in_=wq0)
            nc.scalar.dma_start(out=t0[:, :, P:2 * P], in_=wk0)
            for k in range(2, NCT):
                eng = (nc.sync, nc.scalar, nc.gpsimd)[k % 3]
                eng.dma_start(out=xt_sb[:, k, :], in_=xt_d[k])
            nc.sync.dma_start(out=bo_sb, in_=bo[:, :])
            nc.vector.memset(ones_sb, 1.0)
            load_wv(0)
            load_wqk(1)
            for m in range(NST):
                nc.vector.tensor_copy(
                    v_sb[:, m, :].rearrange("p (h e) -> p h e", e=65)[:, :, 64:65],
                    ones_sb[:, :, None],
                )

            # ---- projection work units ----
            # Chains run in single-bank [128, 512] half-chains so the pjp
            # pool (2 one-bank slots) decouples consecutive chains: the
            # next chain's k=0 WAR lands ~10 units after the previous
            # half's eviction instead of immediately behind it on the DVE.
            def qk_chain_units(ct, which):
                wt = wqk_tiles[ct]
                off = 0 if which == "q" else P
                dst = qt_sb if which == "q" else kt_sb
                box = {}

                def mk(j, k):
                    def f():
                        if k == 0:
                            box[j] = pjp.tile([P, 512], f32, tag="pj",
                                              name="pj_ps")
                        nc.tensor.matmul(
                            box[j][:, 0:512],
                            lhsT=wt[:, k, off:off + P],
                            rhs=xt_sb[:, k, j * 512:(j + 1) * 512],
                            start=(k == 0),
                            stop=(k == NCT - 1),
                        )
                    return f

                def ev(j):
                    def f():
                        nc.vector.tensor_copy(
                            dst[:, ct, j * 512:(j + 1) * 512], box[j]
                        )
                    return f

                units = []
                for j in (0, 1):
                    units += [mk(j, k) for k in range(NCT)]
                    units.append(ev(j))
                return units

            def v_chain_unit(ch, m):
                off, width = V_CHUNKS[ch]
                h0 = off // D
                nh = width // D

                def f():
                    pv = pjp.tile([P, 512], f32, tag="pj", name="pj_pv")
                    wt = wv_tiles[ch]
                    for k in range(NCT):
                        nc.tensor.matmul(
                            pv[:, 0:width],
                            lhsT=xt_sb[:, k, m * P:(m + 1) * P],
                            rhs=wt[:, k, :],
                            start=(k == 0),
                            stop=(k == NCT - 1),
                        )
                    dst = v_sb[:, m, :].rearrange("p (h e) -> p h e", e=65)[
                        :, h0:h0 + nh, 0:64
                    ]
                    nc.vector.tensor_copy(
                        dst, pv[:, 0:width].rearrange("p (h e) -> p h e", e=64)
                    )
                return f

            # ---- attention work units for head pair (2ct, 2ct+1) ----
            def attn_units(cth):
                hA, hB = 2 * cth, 2 * cth + 1
                pts = {}
                po_box = {}

                def sc_unit(qh, kt):
                    def f():
                        ps = psp.tile([P, S], f32, tag="ps", name="ps_sc")
                        nc.tensor.matmul(
                            ps[:, 0:512],
                            lhsT=kt_sb[0:64, cth, kt * P:(kt + 1) * P],
                            rhs=qt_sb[0:64, cth, qh * 512:(qh + 1) * 512],
                            start=True, stop=True,
                        )
                        nc.tensor.matmul(
                            ps[:, 512:1024],
                            lhsT=kt_sb[64:128, cth, kt * P:(kt + 1) * P],
                            rhs=qt_sb[64:128, cth, qh * 512:(qh + 1) * 512],
                            start=True, stop=True,
                        )
                        pt = ptp.tile([P, S], bf16, tag="pt", name="pt_exp")
                        nc.scalar.activation(out=pt, in_=ps, func=EXP, scale=SCALE)
                        pts[(qh, kt)] = pt
                    return f

                def pv_unit(qh, kt):
                    # two single-bank accumulators (poA/poB) so each head's
                    # slot frees as soon as ITS normalize mul is done: the
                    # per-pass po turnaround chain halves vs one [65,1024]
                    def f():
                        if kt == 0:
                            po_box[qh] = (
                                pop.tile([65, 512], f32, tag="po", name="poA"),
                                pop.tile([65, 512], f32, tag="po", name="poB"),
                            )
                        poA, poB = po_box[qh]
                        pt = pts.pop((qh, kt))
                        nc.tensor.matmul(
                            poA[:, 0:512],
                            lhsT=v_sb[:, kt, 65 * hA:65 * hA + 65],
                            rhs=pt[:, 0:512],
                            start=(kt == 0), stop=(kt == NST - 1),
                        )
                        nc.tensor.matmul(
                            poB[:, 0:512],
                            lhsT=v_sb[:, kt, 65 * hB:65 * hB + 65],
                            rhs=pt[:, 512:1024],
                            start=(kt == 0), stop=(kt == NST - 1),
                        )
                    return f

                rb_box = {}

                def norm_a_unit(qh):
                    def f():
                        rbs = []
                        for poX in po_box[qh]:
                            den = rcpp.tile([1, 512], f32, tag="den",
                                            name="den_t")
                            nc.vector.tensor_copy(den, poX[64:65, :])
                            rcp = rcpp.tile([1, 512], f32, tag="rcp",
                                            name="rcp_t")
                            nc.vector.reciprocal_approx_fast(out=rcp, in_=den)
                            rb = rbp.tile([64, 512], f32, tag="rb",
                                          name="rb_t")
                            nc.gpsimd.partition_broadcast(rb, rcp)
                            rbs.append(rb)
                        rb_box[qh] = rbs
                    return f

                def norm_b_unit(qh):
                    # must be emitted before the next pv_unit(kt==0) that
                    # reuses the single po slot (Tile deps are emission-
                    # ordered); splitting lets independent DVE work overlap
                    # the gpsimd broadcast latency.
                    def f():
                        poA, poB = po_box[qh]
                        rbA, rbB = rb_box[qh]
                        nc.vector.tensor_mul(
                            at_sb[0:64, cth, qh * 512:(qh + 1) * 512],
                            poA[0:64, :], rbA,
                        )
                        nc.vector.tensor_mul(
                            at_sb[64:128, cth, qh * 512:(qh + 1) * 512],
                            poB[0:64, :], rbB,
                        )
                    return f

                def sc2_unit(qh, kt2):
                    # two score-pair steps back-to-back: keeps the PE in
                    # 64x128 tiling mode for 4 MMs (mode switches cost
                    # ~100ns of drain each)
                    a, b = sc_unit(qh, kt2), sc_unit(qh, kt2 + 1)

                    def f():
                        a()
                        b()
                    return f

                def pv2_unit(qh, kt2):
                    a, b = pv_unit(qh, kt2), pv_unit(qh, kt2 + 1)

                    def f():
                        a()
                        b()
                    return f

                units = []
                for qh in (0, 1):
                    # all scores first (PV lag 6): the first pv_unit of a
                    # pass reuses the single po slot, whose release waits on
                    # the previous pass's norm chain (gpsimd broadcast +
                    # DVE muls, ~3.4us); deep lag hides that entirely.
                    for kt2 in range(0, NST, 2):
                        units.append(sc2_unit(qh, kt2))
                        if qh == 1 and kt2 == 6:
                            units.append(norm_b_unit(0))
                        if kt2 >= 6:
                            units.append(pv2_unit(qh, kt2 - 6))
                    units.append(pv2_unit(qh, NST - 6))
                    units.append(pv2_unit(qh, NST - 4))
                    units.append(pv2_unit(qh, NST - 2))
                    units.append(norm_a_unit(qh))
                # norm_b(1) is returned as carry: the caller emits it early
                # in the NEXT round, before that pair's first pv_unit.
                return units, norm_b_unit(1)

            def emit_round(attn, proj, front=2):
                if not attn:
                    for u in proj:
                        u()
                    return
                j = 0
                # front-load proj units so the tensor queue is fed while
                # the first scores' inputs settle
                while j < min(front, len(proj)):
                    proj[j]()
                    j += 1
                for i, u in enumerate(attn):
                    u()
                    target = ((i + 1) * len(proj)) // len(attn)
                    while j < target:
                        proj[j]()
                        j += 1
                while j < len(proj):
                    proj[j]()
                    j += 1

            carry_box = [None]

            # ---- round 0 (prologue): Q0, K0, V chunk 0 ----
            proj0 = qk_chain_units(0, "q") + qk_chain_units(0, "k")
            proj0 += [v_chain_unit(0, m) for m in range(NST)]
            emit_round(None, proj0)
            load_wv(1)

            # ---- rounds 1..9 ----
            # V chunk 1 (cols 256:768) m-chains in rounds 1-2;
            # V chunk 2 (cols 768:1280) m-chains in rounds 3-6.
            for r in range(1, NCT):
                proj = []
                if r + 1 < NCT:
                    proj.append(lambda ct=r + 1: load_wqk(ct))
                proj += qk_chain_units(r, "q") + qk_chain_units(r, "k")
                if r in (1, 2):
                    ms = range(0, 4) if r == 1 else range(4, NST)
                    proj += [v_chain_unit(1, m) for m in ms]
                    if r == 2:
                        proj.append(lambda: load_wv(2))
                elif 3 <= r <= 6:
                    proj += [v_chain_unit(2, m) for m in (2 * (r - 3),
                                                          2 * (r - 3) + 1)]
                if r == 9:
                    proj.append(lambda: load_wo(0))
                    proj.append(lambda: load_wo(1))
                    proj.append(lambda: load_wo(2))
                attn, carry = attn_units(r - 1)
                # previous pair's deferred norm_b goes right after the first
                # scores unit (before this pair's first pv_unit reuses po)
                attn.insert(1, carry_box[0]) if carry_box[0] else None
                carry_box[0] = carry
                emit_round(attn, proj)

            # ---- round 10: attention heads 18/19 + O-projection ----
            o_ps = {}

            def o_chain_unit(i, j, kk, pool, ptag):
                def f():
                    if kk == 0:
                        o_ps[(i, j)] = pool.tile([P, 512], f32, tag=ptag,
                                                 name="o_ps_t")
                    nc.tensor.matmul(
                        o_ps[(i, j)][:, 0:512],
                        lhsT=wo_tiles[i][:, kk, :],
                        rhs=at_sb[:, kk, j * 512:(j + 1) * 512],
                        start=(kk == 0),
                        stop=(kk == NCT - 1),
                    )
                return f

            def o_evict_unit(i, j):
                def f():
                    yt_t = ytp.tile([P, 512], f32, tag="yt", name="yt_t")
                    nc.vector.tensor_scalar_add(
                        out=yt_t, in0=o_ps[(i, j)], scalar1=bo_sb[:, i:i + 1]
                    )
                    nc.gpsimd.dma_start(
                        out=yt_d[i][:, j * 512:(j + 1) * 512], in_=yt_t
                    )
                return f

            attn10, carry10 = attn_units(NCT - 1)
            if carry_box[0]:
                attn10.insert(1, carry_box[0])
            # chain 0's two half-chains ride along with the last attention
            # pair as tensor filler (pjp only; psp is still cycling scores
            # tiles).  They may only touch at-tiles 0..8 — tile 9 is
            # produced by this round's attention, and Tile deps are
            # emission-ordered.
            o_early = [lambda: load_wo(3), lambda: load_wo(4), lambda: load_wo(5)]
            o_early += [o_chain_unit(0, 0, kk, pjp, "pj") for kk in range(NCT - 1)]
            o_early += [o_chain_unit(0, 1, kk, pjp, "pj") for kk in range(NCT - 1)]
            emit_round(attn10, o_early, front=6)
            carry10()
            o_chain_unit(0, 0, NCT - 1, pjp, "pj")()
            o_chain_unit(0, 1, NCT - 1, pjp, "pj")()
            o_evict_unit(0, 0)()
            o_evict_unit(0, 1)()
            for i in range(1, NCT):
                if i + 5 < NCT:
                    load_wo(i + 5)
                for j in (0, 1):
                    pool, ptag = (psp, "ps") if i % 2 == 1 else (pjp, "pj")
                    for kk in range(NCT):
                        o_chain_unit(i, j, kk, pool, ptag)()
                    o_evict_unit(i, j)()

    nc.finalize()
    return nc


def kernel(**inputs):
    global last_exec_time_ns
    _install_profile_hook()
    import ml_dtypes
    from concourse.bass_utils import run_bass_kernel_spmd

    bf = ml_dtypes.bfloat16
    hs = np.asarray(inputs["hidden_states"], dtype=np.float32)
    wqt = np.ascontiguousarray(np.asarray(inputs["Wq"], np.float32).T.astype(bf))
    wkt = np.ascontiguousarray(np.asarray(inputs["Wk"], np.float32).T.astype(bf))
    wvt = np.ascontiguousarray(np.asarray(inputs["Wv"], np.float32).T.astype(bf))
    wot = np.ascontiguousarray(np.asarray(inputs["Wo"], np.float32).T.astype(bf))
    bo = np.asarray(inputs["bo"], np.float32).reshape(NCT, P).T.copy()

    if "nc" not in _cache:
        _cache["nc"] = _build()
    nc = _cache["nc"]

    in_maps = [
        {
            "xt": np.ascontiguousarray(hs[b].T.astype(bf)),
            "wq": wqt, "wk": wkt, "wv": wvt, "wo": wot, "bo": bo,
        }
        for b in range(B)
    ]
    res = run_bass_kernel_spmd(nc, in_maps, list(range(B)))
    last_exec_time_ns = res.exec_time_ns
    out = np.stack([res.results[b]["yt"].T for b in range(B)], axis=0)
    return np.ascontiguousarray(out.astype(np.float32))


# revision 17
# speedup vs baseline: 1.0177x; 1.0177x over previous
"""TRN2 Bass kernel for nn_CustomAttnProcessor (B=8, S=1024, C=1280, H=20).

Strategy (v3 — interleaved software pipeline, all-bf16 inputs):
  - Batch-parallel: one batch element per NeuronCore (8 cores), no collectives.
  - Host pre-transposes inputs (xt = X[b].T, w*t = W.T) and casts to bf16;
    kernel emits Y^T in fp32; host transposes back.
  - v1 ran the whole attention phase with the PE HAM clock-gate stuck at
    K=4/8 (1.2 GHz): scores->exp->PV left too many PE idle gaps.  v2/v3
    interleave the Q/K/V projections of c-tile ct+1 with the attention of
    heads 2ct/2ct+1 at matmul granularity, keeping the PE densely busy
    (warm, 2.4 GHz) and hiding ~180us of serial ScalarE exp behind tensor
    work.  The O-projection runs as a dense tail, with its first chain
    pulled into the last attention round as filler (at-tiles 0..8 only —
    tile 9 is produced by that round, and Tile deps are emission-ordered).
  - Head-pair scores are emitted back-to-back so bass's auto tile_position
    ((0,0) for even heads, (64,0) for odd heads, tile_size (64,128)) lets
    the PE overlap the two 64-contraction matmuls (2x row tiling).
  - q is processed in halves of 512 so the PV accumulators for a head pair
    fit in 2 PSUM banks ([65, 512+512]); scores pair tiles are [128, 1024]
    (2 banks, one exp ACTIVATE of N=1024 covers both heads).
  - All matmul operands bf16 (inputs rounded on host; rel err ~5e-3 vs the
    2e-2 gate), accumulation fp32 in PSUM.  bf16 halves HBM traffic and
    SBUF footprint vs fp32r and enables FWL weight loads.
  - PV uses the ones-augmented-V trick: stationary [128, 65] whose 65th
    column accumulates the softmax denominator for free.
"""

import sys
import types

import numpy as np

B, S, C, H = 8, 1024, 1280, 20
D = C // H              # 64 head dim
P = 128
NCT = C // P            # 10 c-tiles
NST = S // P            # 8 s-tiles
SCALE = float(D) ** -0.5

# V-projection chunks: (col_offset, width) and the rounds their m-chains run
V_CHUNKS = [(0, 256), (256, 512), (768, 512)]

_cache = {}

last_exec_time_ns = None


def _install_profile_hook():
    """antenv.axon_hooks is absent in this container; reconstruct it from
    trn_agent_boot so BASS_TRACE=1 profiling works."""
    if "antenv.axon_hooks" in sys.modules:
        return
    mod = types.ModuleType("antenv.axon_hooks")
    state = {"hook": None}
    mod.set_axon_ntff_profile_hook = lambda h: state.update(hook=h)
    mod.get_axon_ntff_profile_hook = lambda: state["hook"]
    sys.modules["antenv.axon_hooks"] = mod
    try:
        from trn_agent_boot.trn_boot import _ntff_profile_via_ctypes

        hook = _ntff_profile_via_ctypes("/opt/axon/libaxon_pjrt.so")
        if hook is not None:
            mod.set_axon_ntff_profile_hook(hook)
    except Exception:
        pass


def _build():
    import concourse.mybir as mybir
    from concourse import bacc
    from concourse.tile import TileContext

    f32 = mybir.dt.float32
    bf16 = mybir.dt.bfloat16
    EXP = mybir.ActivationFunctionType.Exp

    nc = bacc.Bacc()
    xt = nc.declare_dram_parameter("xt", [C, S], bf16, isOutput=False)
    wq = nc.declare_dram_parameter("wq", [C, C], bf16, isOutput=False)
    wk = nc.declare_dram_parameter("wk", [C, C], bf16, isOutput=False)
    wv = nc.declare_dram_parameter("wv", [C, C], bf16, isOutput=False)
    wo = nc.declare_dram_parameter("wo", [C, C], bf16, isOutput=False)
    bo = nc.declare_dram_parameter("bo", [P, NCT], f32, isOutput=False)
    yt = nc.declare_dram_parameter("yt", [C, S], f32, isOutput=True)

    with TileContext(nc) as tc:
        with (
            tc.tile_pool(name="big", bufs=1) as big,
            tc.tile_pool(name="wqk", bufs=2) as wqkp,
            tc.tile_pool(name="wvp", bufs=2) as wvp,
            tc.tile_pool(name="wop", bufs=6) as wop,
            tc.tile_pool(name="ptp", bufs=10) as ptp,
            tc.tile_pool(name="rbp", bufs=2) as rbp,
            tc.tile_pool(name="rcpp", bufs=1) as rcpp,
            tc.tile_pool(name="ytp", bufs=2) as ytp,
            tc.tile_pool(name="smt", bufs=1) as smt,
            tc.tile_pool(name="psp", bufs=2, space="PSUM") as psp,
            tc.tile_pool(name="pop", bufs=1, space="PSUM") as pop,
            tc.tile_pool(name="pjp", bufs=2, space="PSUM") as pjp,
        ):
            # ---- persistent tiles ----
            xt_sb = big.tile([P, NCT, S], bf16, tag="xt")
            qt_sb = big.tile([P, NCT, S], bf16, tag="qt")
            kt_sb = big.tile([P, NCT, S], bf16, tag="kt")
            v_sb = big.tile([P, NST, H * 65], bf16, tag="v")
            at_sb = big.tile([P, NCT, S], bf16, tag="at")
            bo_sb = smt.tile([P, NCT], f32, tag="bo")
            ones_sb = smt.tile([P, H], bf16, tag="ones")

            xt_d = xt.rearrange("(t p) s -> t p s", p=P)
            yt_d = yt.rearrange("(t p) s -> t p s", p=P)

            # ---- weight staging ----
            wqk_tiles = {}

            def load_wqk(ct):
                t = wqkp.tile([P, NCT, 2 * P], bf16, tag="wqk", name="wqk_t")
                nc.sync.dma_start(
                    out=t[:, :, 0:P],
                    in_=wq[:, ct * P:(ct + 1) * P].rearrange("(t p) o -> p t o", p=P),
                )
                nc.sync.dma_start(
                    out=t[:, :, P:2 * P],
                    in_=wk[:, ct * P:(ct + 1) * P].rearrange("(t p) o -> p t o", p=P),
                )
                wqk_tiles[ct] = t

            wv_tiles = {}

            def load_wv(ch):
                off, width = V_CHUNKS[ch]
                t = wvp.tile([P, NCT, width], bf16, tag="wv", name="wv_t")
                nc.sync.dma_start(
                    out=t,
                    in_=wv[:, off:off + width].rearrange("(t p) o -> p t o", p=P),
                )
                wv_tiles[ch] = t

            wo_tiles = {}

            def load_wo(i):
                t = wop.tile([P, NCT, P], bf16, tag="wo", name="wo_t")
                nc.scalar.dma_start(
                    out=t,
                    in_=wo[:, i * P:(i + 1) * P].rearrange("(t p) o -> p t o", p=P),
                )
                wo_tiles[i] = t

            # ---- prologue DMAs, in consumption order, two queues ----
            # Per-DMA queue-issue overhead is ~0.9us, so split across the
            # sync and scalar (Activation) DGE queues and order by first use:
            # xt0/xt1, then the Q/K weight halves, then remaining xt chunks.
            t0 = wqkp.tile([P, NCT, 2 * P], bf16, tag="wqk", name="wqk_t")
            wqk_tiles[0] = t0
            wq0 = wq[:, 0:P].rearrange("(t p) o -> p t o", p=P)
            wk0 = wk[:, 0:P].rearrange("(t p) o -> p t o", p=P)
            nc.sync.dma_start(out=xt_sb[:, 0, :], in_=xt_d[0])
            nc.scalar.dma_start(out=xt_sb[:, 1, :], in_=xt_d[1])
            nc.sync.dma_start(out=t0[:, :, 0:P], in_=wq0)
            nc.scalar.dma_start(out=t0[:, :, P:2 * P], in_=wk0)
            for k in range(2, NCT):
                eng = (nc.sync, nc.scalar, nc.gpsimd)[k % 3]
                eng.dma_start(out=xt_sb[:, k, :], in_=xt_d[k])
            nc.sync.dma_start(out=bo_sb, in_=bo[:, :])
            nc.vector.memset(ones_sb, 1.0)
            load_wv(0)
            load_wqk(1)
            for m in range(NST):
                nc.vector.tensor_copy(
                    v_sb[:, m, :].rearrange("p (h e) -> p h e", e=65)[:, :, 64:65],
                    ones_sb[:, :, None],
                )

            # ---- projection work units ----
            # Chains run in single-bank [128, 512] half-chains so the pjp
            # pool (2 one-bank slots) decouples consecutive chains: the
            # next chain's k=0 WAR lands ~10 units after the previous
            # half's eviction instead of immediately behind it on the DVE.
            def qk_chain_units(ct, which):
                wt = wqk_tiles[ct]
                off = 0 if which == "q" else P
                dst = qt_sb if which == "q" else kt_sb
                box = {}

                def mk(j, k):
                    def f():
                        if k == 0:
                            box[j] = pjp.tile([P, 512], f32, tag="pj",
                                              name="pj_ps")
                        nc.tensor.matmul(
                            box[j][:, 0:512],
                            lhsT=wt[:, k, off:off + P],
                            rhs=xt_sb[:, k, j * 512:(j + 1) * 512],
                            start=(k == 0),
                            stop=(k == NCT - 1),
                        )
                    return f

                def ev(j):
                    def f():
                        nc.vector.tensor_copy(
                            dst[:, ct, j * 512:(j + 1) * 512], box[j]
                        )
                    return f

                units = []
                for j in (0, 1):
                    units += [mk(j, k) for k in range(NCT)]
                    units.append(ev(j))
                return units

            def v_chain_unit(ch, m):
                off, width = V_CHUNKS[ch]
                h0 = off // D
                nh = width // D

                def f():
                    pv = pjp.tile([P, 512], f32, tag="pj", name="pj_pv")
                    wt = wv_tiles[ch]
                    for k in range(NCT):
                        nc.tensor.matmul(
                            pv[:, 0:width],
                            lhsT=xt_sb[:, k, m * P:(m + 1) * P],
                            rhs=wt[:, k, :],
                            start=(k == 0),
                            stop=(k == NCT - 1),
                        )
                    dst = v_sb[:, m, :].rearrange("p (h e) -> p h e", e=65)[
                        :, h0:h0 + nh, 0:64
                    ]
                    nc.vector.tensor_copy(
                        dst, pv[:, 0:width].rearrange("p (h e) -> p h e", e=64)
                    )
                return f

            # ---- attention work units for head pair (2ct, 2ct+1) ----
            def attn_units(cth):
                hA, hB = 2 * cth, 2 * cth + 1
                pts = {}
                po_box = {}

                def sc_unit(qh, kt):
                    def f():
                        ps = psp.tile([P, S], f32, tag="ps", name="ps_sc")
                        nc.tensor.matmul(
                            ps[:, 0:512],
                            lhsT=kt_sb[0:64, cth, kt * P:(kt + 1) * P],
                            rhs=qt_sb[0:64, cth, qh * 512:(qh + 1) * 512],
                            start=True, stop=True,
                        )
                        nc.tensor.matmul(
                            ps[:, 512:1024],
                            lhsT=kt_sb[64:128, cth, kt * P:(kt + 1) * P],
                            rhs=qt_sb[64:128, cth, qh * 512:(qh + 1) * 512],
                            start=True, stop=True,
                        )
                        pt = ptp.tile([P, S], bf16, tag="pt", name="pt_exp")
                        nc.scalar.activation(out=pt, in_=ps, func=EXP, scale=SCALE)
                        pts[(qh, kt)] = pt
                    return f

                def pv_unit(qh, kt):
                    def f():
                        if kt == 0:
                            po_box[qh] = pop.tile([65, S], f32, tag="po",
                                                  name="po_acc")
                        po_t = po_box[qh]
                        pt = pts.pop((qh, kt))
                        nc.tensor.matmul(
                            po_t[:, 0:512],
                            lhsT=v_sb[:, kt, 65 * hA:65 * hA + 65],
                            rhs=pt[:, 0:512],
                            start=(kt == 0), stop=(kt == NST - 1),
                        )
                        nc.tensor.matmul(
                            po_t[:, 512:1024],
                            lhsT=v_sb[:, kt, 65 * hB:65 * hB + 65],
                            rhs=pt[:, 512:1024],
                            start=(kt == 0), stop=(kt == NST - 1),
                        )
                    return f

                rb_box = {}

                def norm_a_unit(qh):
                    def f():
                        po_t = po_box[qh]
                        den = rcpp.tile([1, S], f32, tag="den", name="den_t")
                        nc.vector.tensor_copy(den, po_t[64:65, :])
                        rcp = rcpp.tile([1, S], f32, tag="rcp", name="rcp_t")
                        nc.vector.reciprocal_approx_fast(out=rcp, in_=den)
                        rb = rbp.tile([64, S], f32, tag="rb", name="rb_t")
                        nc.gpsimd.partition_broadcast(rb, rcp)
                        rb_box[qh] = rb
                    return f

                def norm_b_unit(qh):
                    # must be emitted before the next pv_unit(kt==0) that
                    # reuses the single po slot (Tile deps are emission-
                    # ordered); splitting lets independent DVE work overlap
                    # the gpsimd broadcast latency.
                    def f():
                        po_t = po_box[qh]
                        rb = rb_box[qh]
                        nc.vector.tensor_mul(
                            at_sb[0:64, cth, qh * 512:(qh + 1) * 512],
                            po_t[0:64, 0:512], rb[:, 0:512],
                        )
                        nc.vector.tensor_mul(
                            at_sb[64:128, cth, qh * 512:(qh + 1) * 512],
                            po_t[0:64, 512:1024], rb[:, 512:1024],
                        )
                    return f

                def sc2_unit(qh, kt2):
                    # two score-pair steps back-to-back: keeps the PE in
                    # 64x128 tiling mode for 4 MMs (mode switches cost
                    # ~100ns of drain each)
                    a, b = sc_unit(qh, kt2), sc_unit(qh, kt2 + 1)

                    def f():
                        a()
                        b()
                    return f

                def pv2_unit(qh, kt2):
                    a, b = pv_unit(qh, kt2), pv_unit(qh, kt2 + 1)

                    def f():
                        a()
                        b()
                    return f

                units = []
                for qh in (0, 1):
                    # all scores first (PV lag 6): the first pv_unit of a
                    # pass reuses the single po slot, whose release waits on
                    # the previous pass's norm chain (gpsimd broadcast +
                    # DVE muls, ~3.4us); deep lag hides that entirely.
                    for kt2 in range(0, NST, 2):
                        units.append(sc2_unit(qh, kt2))
                        if qh == 1 and kt2 == 6:
                            units.append(norm_b_unit(0))
                        if kt2 >= 6:
                            units.append(pv2_unit(qh, kt2 - 6))
                    units.append(pv2_unit(qh, NST - 6))
                    units.append(pv2_unit(qh, NST - 4))
                    units.append(pv2_unit(qh, NST - 2))
                    units.append(norm_a_unit(qh))
                # norm_b(1) is returned as carry: the caller emits it early
                # in the NEXT round, before that pair's first pv_unit.
                return units, norm_b_unit(1)

            def emit_round(attn, proj, front=2):
                if not attn:
                    for u in proj:
                        u()
                    return
                j = 0
                # front-load proj units so the tensor queue is fed while
                # the first scores' inputs settle
                while j < min(front, len(proj)):
                    proj[j]()
                    j += 1
                for i, u in enumerate(attn):
                    u()
                    target = ((i + 1) * len(proj)) // len(attn)
                    while j < target:
                        proj[j]()
                        j += 1
                while j < len(proj):
                    proj[j]()
                    j += 1

            carry_box = [None]

            # ---- round 0 (prologue): Q0, K0, V chunk 0 ----
            proj0 = qk_chain_units(0, "q") + qk_chain_units(0, "k")
            proj0 += [v_chain_unit(0, m) for m in range(NST)]
            emit_round(None, proj0)
            load_wv(1)

            # ---- rounds 1..9 ----
            # V chunk 1 (cols 256:768) m-chains in rounds 1-2;
            # V chunk 2 (cols 768:1280) m-chains in rounds 3-6.
            for r in range(1, NCT):
                proj = []
                if r + 1 < NCT:
                    proj.append(lambda ct=r + 1: load_wqk(ct))
                proj += qk_chain_units(r, "q") + qk_chain_units(r, "k")
                if r in (1, 2):
                    ms = range(0, 4) if r == 1 else range(4, NST)
                    proj += [v_chain_unit(1, m) for m in ms]
                    if r == 2:
                        proj.append(lambda: load_wv(2))
                elif 3 <= r <= 6:
                    proj += [v_chain_unit(2, m) for m in (2 * (r - 3),
                                                          2 * (r - 3) + 1)]
                if r == 9:
                    proj.append(lambda: load_wo(0))
                    proj.append(lambda: load_wo(1))
                    proj.append(lambda: load_wo(2))
                attn, carry = attn_units(r - 1)
                # previous pair's deferred norm_b goes right after the first
                # scores unit (before this pair's first pv_unit reuses po)
                attn.insert(1, carry_box[0]) if carry_box[0] else None
                carry_box[0] = carry
                emit_round(attn, proj)

            # ---- round 10: attention heads 18/19 + O-projection ----
            o_ps = {}

            def o_chain_unit(i, j, kk, pool, ptag):
                def f():
                    if kk == 0:
                        o_ps[(i, j)] = pool.tile([P, 512], f32, tag=ptag,
                                                 name="o_ps_t")
                    nc.tensor.matmul(
                        o_ps[(i, j)][:, 0:512],
                        lhsT=wo_tiles[i][:, kk, :],
                        rhs=at_sb[:, kk, j * 512:(j + 1) * 512],
                        start=(kk == 0),
                        stop=(kk == NCT - 1),
                    )
                return f

            def o_evict_unit(i, j):
                def f():
                    yt_t = ytp.tile([P, 512], f32, tag="yt", name="yt_t")
                    nc.vector.tensor_scalar_add(
                        out=yt_t, in0=o_ps[(i, j)], scalar1=bo_sb[:, i:i + 1]
                    )
                    nc.gpsimd.dma_start(
                        out=yt_d[i][:, j * 512:(j + 1) * 512], in_=yt_t
                    )
                return f

            attn10, carry10 = attn_units(NCT - 1)
            if carry_box[0]:
                attn10.insert(1, carry_box[0])
            # chain 0's two half-chains ride along with the last attention
            # pair as tensor filler (pjp only; psp is still cycling scores
            # tiles).  They may only touch at-tiles 0..8 — tile 9 is
            # produced by this round's attention, and Tile deps are
            # emission-ordered.
            o_early = [lambda: load_wo(3), lambda: load_wo(4), lambda: load_wo(5)]
            o_early += [o_chain_unit(0, 0, kk, pjp, "pj") for kk in range(NCT - 1)]
            o_early += [o_chain_unit(0, 1, kk, pjp, "pj") for kk in range(NCT - 1)]
            emit_round(attn10, o_early, front=6)
            carry10()
            o_chain_unit(0, 0, NCT - 1, pjp, "pj")()
            o_chain_unit(0, 1, NCT - 1, pjp, "pj")()
            o_evict_unit(0, 0)()
            o_evict_unit(0, 1)()
            for i in range(1, NCT):
                if i + 5 < NCT:
                    load_wo(i + 5)
                for j in (0, 1):
                    pool, ptag = (psp, "ps") if i % 2 == 1 else (pjp, "pj")
                    for kk in range(NCT):
                        o_chain_unit(i, j, kk, pool, ptag)()
                    o_evict_unit(i, j)()

    nc.finalize()
    return nc


def kernel(**inputs):
    global last_exec_time_ns
    _install_profile_hook()
    import ml_dtypes
    from concourse.bass_utils import run_bass_kernel_spmd

    bf = ml_dtypes.bfloat16
    hs = np.asarray(inputs["hidden_states"], dtype=np.float32)
    wqt = np.ascontiguousarray(np.asarray(inputs["Wq"], np.float32).T.astype(bf))
    wkt = np.ascontiguousarray(np.asarray(inputs["Wk"], np.float32).T.astype(bf))
    wvt = np.ascontiguousarray(np.asarray(inputs["Wv"], np.float32).T.astype(bf))
    wot = np.ascontiguousarray(np.asarray(inputs["Wo"], np.float32).T.astype(bf))
    bo = np.asarray(inputs["bo"], np.float32).reshape(NCT, P).T.copy()

    if "nc" not in _cache:
        _cache["nc"] = _build()
    nc = _cache["nc"]

    in_maps = [
        {
            "xt": np.ascontiguousarray(hs[b].T.astype(bf)),
            "wq": wqt, "wk": wkt, "wv": wvt, "wo": wot, "bo": bo,
        }
        for b in range(B)
    ]
    res = run_bass_kernel_spmd(nc, in_maps, list(range(B)))
    last_exec_time_ns = res.exec_time_ns
    out = np.stack([res.results[b]["yt"].T for b in range(B)], axis=0)
    return np.ascontiguousarray(out.astype(np.float32))


# revision 18
# speedup vs baseline: 1.2172x; 1.1961x over previous
"""TRN2 Bass kernel for nn_CustomAttnProcessor (B=8, S=1024, C=1280, H=20).

Strategy (v3 — interleaved software pipeline, all-bf16 inputs):
  - Batch-parallel: one batch element per NeuronCore (8 cores), no collectives.
  - Host pre-transposes inputs (xt = X[b].T, w*t = W.T) and casts to bf16;
    kernel emits Y^T in fp32; host transposes back.
  - v1 ran the whole attention phase with the PE HAM clock-gate stuck at
    K=4/8 (1.2 GHz): scores->exp->PV left too many PE idle gaps.  v2/v3
    interleave the Q/K/V projections of c-tile ct+1 with the attention of
    heads 2ct/2ct+1 at matmul granularity, keeping the PE densely busy
    (warm, 2.4 GHz) and hiding ~180us of serial ScalarE exp behind tensor
    work.  The O-projection runs as a dense tail, with its first chain
    pulled into the last attention round as filler (at-tiles 0..8 only —
    tile 9 is produced by that round, and Tile deps are emission-ordered).
  - Head-pair scores are emitted back-to-back so bass's auto tile_position
    ((0,0) for even heads, (64,0) for odd heads, tile_size (64,128)) lets
    the PE overlap the two 64-contraction matmuls (2x row tiling).
  - q is processed in halves of 512 so the PV accumulators for a head pair
    fit in 2 PSUM banks ([65, 512+512]); scores pair tiles are [128, 1024]
    (2 banks, one exp ACTIVATE of N=1024 covers both heads).
  - All matmul operands bf16 (inputs rounded on host; rel err ~5e-3 vs the
    2e-2 gate), accumulation fp32 in PSUM.  bf16 halves HBM traffic and
    SBUF footprint vs fp32r and enables FWL weight loads.
  - PV uses the ones-augmented-V trick: stationary [128, 65] whose 65th
    column accumulates the softmax denominator for free.
"""

import sys
import types

import numpy as np

B, S, C, H = 8, 1024, 1280, 20
D = C // H              # 64 head dim
P = 128
NCT = C // P            # 10 c-tiles
NST = S // P            # 8 s-tiles
SCALE = float(D) ** -0.5

# V-projection chunks: (col_offset, width) and the rounds their m-chains run
V_CHUNKS = [(0, 256), (256, 512), (768, 512)]

_cache = {}

last_exec_time_ns = None


def _install_profile_hook():
    """antenv.axon_hooks is absent in this container; reconstruct it from
    trn_agent_boot so BASS_TRACE=1 profiling works."""
    if "antenv.axon_hooks" in sys.modules:
        return
    mod = types.ModuleType("antenv.axon_hooks")
    state = {"hook": None}
    mod.set_axon_ntff_profile_hook = lambda h: state.update(hook=h)
    mod.get_axon_ntff_profile_hook = lambda: state["hook"]
    sys.modules["antenv.axon_hooks"] = mod
    try:
        from trn_agent_boot.trn_boot import _ntff_profile_via_ctypes

        hook = _ntff_profile_via_ctypes("/opt/axon/libaxon_pjrt.so")
        if hook is not None:
            mod.set_axon_ntff_profile_hook(hook)
    except Exception:
        pass


def _build():
    import concourse.mybir as mybir
    from concourse import bacc
    from concourse.tile import TileContext

    f32 = mybir.dt.float32
    bf16 = mybir.dt.bfloat16
    EXP = mybir.ActivationFunctionType.Exp

    nc = bacc.Bacc()
    xt = nc.declare_dram_parameter("xt", [C, S], bf16, isOutput=False)
    wq = nc.declare_dram_parameter("wq", [C, C], bf16, isOutput=False)
    wk = nc.declare_dram_parameter("wk", [C, C], bf16, isOutput=False)
    wv = nc.declare_dram_parameter("wv", [C, C], bf16, isOutput=False)
    wo = nc.declare_dram_parameter("wo", [C, C], bf16, isOutput=False)
    bo = nc.declare_dram_parameter("bo", [P, NCT], f32, isOutput=False)
    yt = nc.declare_dram_parameter("yt", [C, S], f32, isOutput=True)

    with TileContext(nc) as tc:
        with (
            tc.tile_pool(name="big", bufs=1) as big,
            tc.tile_pool(name="wqk", bufs=2) as wqkp,
            tc.tile_pool(name="wvp", bufs=2) as wvp,
            tc.tile_pool(name="wop", bufs=6) as wop,
            tc.tile_pool(name="ptp", bufs=10) as ptp,
            tc.tile_pool(name="rbp", bufs=2) as rbp,
            tc.tile_pool(name="rcpp", bufs=1) as rcpp,
            tc.tile_pool(name="ytp", bufs=4) as ytp,
            tc.tile_pool(name="smt", bufs=1) as smt,
            tc.tile_pool(name="psp", bufs=2, space="PSUM") as psp,
            tc.tile_pool(name="pop", bufs=1, space="PSUM") as pop,
            tc.tile_pool(name="pjp", bufs=2, space="PSUM") as pjp,
        ):
            # ---- persistent tiles ----
            xt_sb = big.tile([P, NCT, S], bf16, tag="xt")
            qt_sb = big.tile([P, NCT, S], bf16, tag="qt")
            kt_sb = big.tile([P, NCT, S], bf16, tag="kt")
            v_sb = big.tile([P, NST, H * 65], bf16, tag="v")
            at_sb = big.tile([P, NCT, S], bf16, tag="at")
            bo_sb = smt.tile([P, NCT], f32, tag="bo")
            ones_sb = smt.tile([P, H], bf16, tag="ones")

            xt_d = xt.rearrange("(t p) s -> t p s", p=P)
            yt_d = yt.rearrange("(t p) s -> t p s", p=P)

            # ---- weight staging ----
            wqk_tiles = {}

            def load_wqk(ct):
                t = wqkp.tile([P, NCT, 2 * P], bf16, tag="wqk", name="wqk_t")
                nc.sync.dma_start(
                    out=t[:, :, 0:P],
                    in_=wq[:, ct * P:(ct + 1) * P].rearrange("(t p) o -> p t o", p=P),
                )
                nc.sync.dma_start(
                    out=t[:, :, P:2 * P],
                    in_=wk[:, ct * P:(ct + 1) * P].rearrange("(t p) o -> p t o", p=P),
                )
                wqk_tiles[ct] = t

            wv_tiles = {}

            def load_wv(ch):
                off, width = V_CHUNKS[ch]
                t = wvp.tile([P, NCT, width], bf16, tag="wv", name="wv_t")
                nc.sync.dma_start(
                    out=t,
                    in_=wv[:, off:off + width].rearrange("(t p) o -> p t o", p=P),
                )
                wv_tiles[ch] = t

            wo_tiles = {}

            def load_wo(i):
                t = wop.tile([P, NCT, P], bf16, tag="wo", name="wo_t")
                nc.scalar.dma_start(
                    out=t,
                    in_=wo[:, i * P:(i + 1) * P].rearrange("(t p) o -> p t o", p=P),
                )
                wo_tiles[i] = t

            # ---- prologue DMAs, in consumption order, two queues ----
            # Per-DMA queue-issue overhead is ~0.9us, so split across the
            # sync and scalar (Activation) DGE queues and order by first use:
            # xt0/xt1, then the Q/K weight halves, then remaining xt chunks.
            t0 = wqkp.tile([P, NCT, 2 * P], bf16, tag="wqk", name="wqk_t")
            wqk_tiles[0] = t0
            wq0 = wq[:, 0:P].rearrange("(t p) o -> p t o", p=P)
            wk0 = wk[:, 0:P].rearrange("(t p) o -> p t o", p=P)
            nc.sync.dma_start(out=xt_sb[:, 0, :], in_=xt_d[0])
            nc.scalar.dma_start(out=xt_sb[:, 1, :], in_=xt_d[1])
            nc.sync.dma_start(out=t0[:, :, 0:P], in_=wq0)
            nc.scalar.dma_start(out=t0[:, :, P:2 * P], in_=wk0)
            for k in range(2, NCT):
                eng = (nc.sync, nc.scalar, nc.gpsimd)[k % 3]
                eng.dma_start(out=xt_sb[:, k, :], in_=xt_d[k])
            nc.gpsimd.dma_start(out=bo_sb, in_=bo[:, :])
            nc.vector.memset(ones_sb, 1.0)
            # wv0 on the scalar queue so it lands before the first V chain
            # needs it (the sync queue is busy with xt chunks + wqk1)
            t = wvp.tile([P, NCT, V_CHUNKS[0][1]], bf16, tag="wv", name="wv_t")
            nc.scalar.dma_start(
                out=t, in_=wv[:, 0:V_CHUNKS[0][1]].rearrange("(t p) o -> p t o", p=P)
            )
            wv_tiles[0] = t
            load_wqk(1)
            for m in range(NST):
                nc.vector.tensor_copy(
                    v_sb[:, m, :].rearrange("p (h e) -> p h e", e=65)[:, :, 64:65],
                    ones_sb[:, :, None],
                )

            # ---- projection work units ----
            # Chains run in single-bank [128, 512] half-chains so the pjp
            # pool (2 one-bank slots) decouples consecutive chains: the
            # next chain's k=0 WAR lands ~10 units after the previous
            # half's eviction instead of immediately behind it on the DVE.
            def qk_chain_units(ct, which):
                wt = wqk_tiles[ct]
                off = 0 if which == "q" else P
                dst = qt_sb if which == "q" else kt_sb
                box = {}

                def mk(j, k):
                    def f():
                        if k == 0:
                            box[j] = pjp.tile([P, 512], f32, tag="pj",
                                              name="pj_ps")
                        nc.tensor.matmul(
                            box[j][:, 0:512],
                            lhsT=wt[:, k, off:off + P],
                            rhs=xt_sb[:, k, j * 512:(j + 1) * 512],
                            start=(k == 0),
                            stop=(k == NCT - 1),
                        )
                    return f

                def ev(j):
                    def f():
                        nc.vector.tensor_copy(
                            dst[:, ct, j * 512:(j + 1) * 512], box[j]
                        )
                    return f

                units = []
                for j in (0, 1):
                    units += [mk(j, k) for k in range(NCT)]
                    units.append(ev(j))
                return units

            def v_chain_unit(ch, m):
                off, width = V_CHUNKS[ch]
                h0 = off // D
                nh = width // D

                def f():
                    pv = pjp.tile([P, 512], f32, tag="pj", name="pj_pv")
                    wt = wv_tiles[ch]
                    for k in range(NCT):
                        nc.tensor.matmul(
                            pv[:, 0:width],
                            lhsT=xt_sb[:, k, m * P:(m + 1) * P],
                            rhs=wt[:, k, :],
                            start=(k == 0),
                            stop=(k == NCT - 1),
                        )
                    dst = v_sb[:, m, :].rearrange("p (h e) -> p h e", e=65)[
                        :, h0:h0 + nh, 0:64
                    ]
                    nc.vector.tensor_copy(
                        dst, pv[:, 0:width].rearrange("p (h e) -> p h e", e=64)
                    )
                return f

            # ---- attention work units for head pair (2ct, 2ct+1) ----
            def attn_units(cth):
                hA, hB = 2 * cth, 2 * cth + 1
                pts = {}
                po_box = {}

                def sc_unit(qh, kt):
                    def f():
                        ps = psp.tile([P, S], f32, tag="ps", name="ps_sc")
                        nc.tensor.matmul(
                            ps[:, 0:512],
                            lhsT=kt_sb[0:64, cth, kt * P:(kt + 1) * P],
                            rhs=qt_sb[0:64, cth, qh * 512:(qh + 1) * 512],
                            start=True, stop=True,
                        )
                        nc.tensor.matmul(
                            ps[:, 512:1024],
                            lhsT=kt_sb[64:128, cth, kt * P:(kt + 1) * P],
                            rhs=qt_sb[64:128, cth, qh * 512:(qh + 1) * 512],
                            start=True, stop=True,
                        )
                        pt = ptp.tile([P, S], bf16, tag="pt", name="pt_exp")
                        nc.scalar.activation(out=pt, in_=ps, func=EXP, scale=SCALE)
                        pts[(qh, kt)] = pt
                    return f

                def pv_unit(qh, kt):
                    def f():
                        if kt == 0:
                            po_box[qh] = pop.tile([65, S], f32, tag="po",
                                                  name="po_acc")
                        po_t = po_box[qh]
                        pt = pts.pop((qh, kt))
                        nc.tensor.matmul(
                            po_t[:, 0:512],
                            lhsT=v_sb[:, kt, 65 * hA:65 * hA + 65],
                            rhs=pt[:, 0:512],
                            start=(kt == 0), stop=(kt == NST - 1),
                        )
                        nc.tensor.matmul(
                            po_t[:, 512:1024],
                            lhsT=v_sb[:, kt, 65 * hB:65 * hB + 65],
                            rhs=pt[:, 512:1024],
                            start=(kt == 0), stop=(kt == NST - 1),
                        )
                    return f

                rb_box = {}

                def norm_a_unit(qh):
                    def f():
                        po_t = po_box[qh]
                        den = rcpp.tile([1, S], f32, tag="den", name="den_t")
                        nc.vector.tensor_copy(den, po_t[64:65, :])
                        rcp = rcpp.tile([1, S], f32, tag="rcp", name="rcp_t")
                        nc.vector.reciprocal_approx_fast(out=rcp, in_=den)
                        rb = rbp.tile([64, S], f32, tag="rb", name="rb_t")
                        nc.gpsimd.partition_broadcast(rb, rcp)
                        rb_box[qh] = rb
                    return f

                def norm_b_unit(qh):
                    # must be emitted before the next pv_unit(kt==0) that
                    # reuses the single po slot (Tile deps are emission-
                    # ordered); splitting lets independent DVE work overlap
                    # the gpsimd broadcast latency.
                    def f():
                        po_t = po_box[qh]
                        rb = rb_box[qh]
                        nc.vector.tensor_mul(
                            at_sb[0:64, cth, qh * 512:(qh + 1) * 512],
                            po_t[0:64, 0:512], rb[:, 0:512],
                        )
                        nc.vector.tensor_mul(
                            at_sb[64:128, cth, qh * 512:(qh + 1) * 512],
                            po_t[0:64, 512:1024], rb[:, 512:1024],
                        )
                    return f

                def sc2_unit(qh, kt2):
                    # two score-pair steps back-to-back: keeps the PE in
                    # 64x128 tiling mode for 4 MMs (mode switches cost
                    # ~100ns of drain each)
                    a, b = sc_unit(qh, kt2), sc_unit(qh, kt2 + 1)

                    def f():
                        a()
                        b()
                    return f

                def pv2_unit(qh, kt2):
                    a, b = pv_unit(qh, kt2), pv_unit(qh, kt2 + 1)

                    def f():
                        a()
                        b()
                    return f

                units = []
                for qh in (0, 1):
                    # all scores first (PV lag 6): the first pv_unit of a
                    # pass reuses the single po slot, whose release waits on
                    # the previous pass's norm chain (gpsimd broadcast +
                    # DVE muls, ~3.4us); deep lag hides that entirely.
                    for kt2 in range(0, NST, 2):
                        units.append(sc2_unit(qh, kt2))
                        if qh == 1 and kt2 == 6:
                            units.append(norm_b_unit(0))
                        if kt2 >= 6:
                            units.append(pv2_unit(qh, kt2 - 6))
                    units.append(pv2_unit(qh, NST - 6))
                    units.append(pv2_unit(qh, NST - 4))
                    units.append(pv2_unit(qh, NST - 2))
                    units.append(norm_a_unit(qh))
                # norm_b(1) is returned as carry: the caller emits it early
                # in the NEXT round, before that pair's first pv_unit.
                return units, norm_b_unit(1)

            def emit_round(attn, proj, front=2):
                if not attn:
                    for u in proj:
                        u()
                    return
                j = 0
                # front-load proj units so the tensor queue is fed while
                # the first scores' inputs settle
                while j < min(front, len(proj)):
                    proj[j]()
                    j += 1
                for i, u in enumerate(attn):
                    u()
                    target = ((i + 1) * len(proj)) // len(attn)
                    while j < target:
                        proj[j]()
                        j += 1
                while j < len(proj):
                    proj[j]()
                    j += 1

            carry_box = [None]

            # ---- round 0 (prologue): Q0, K0, V chunk 0 ----
            proj0 = qk_chain_units(0, "q") + qk_chain_units(0, "k")
            proj0 += [v_chain_unit(0, m) for m in range(NST)]
            emit_round(None, proj0)
            load_wv(1)

            # ---- rounds 1..9 ----
            # V chunk 1 (cols 256:768) m-chains in rounds 1-2;
            # V chunk 2 (cols 768:1280) m-chains in rounds 3-6.
            for r in range(1, NCT):
                proj = []
                if r + 1 < NCT:
                    proj.append(lambda ct=r + 1: load_wqk(ct))
                proj += qk_chain_units(r, "q") + qk_chain_units(r, "k")
                if r in (1, 2):
                    ms = range(0, 4) if r == 1 else range(4, NST)
                    proj += [v_chain_unit(1, m) for m in ms]
                    if r == 2:
                        proj.append(lambda: load_wv(2))
                elif 3 <= r <= 6:
                    proj += [v_chain_unit(2, m) for m in (2 * (r - 3),
                                                          2 * (r - 3) + 1)]
                if r == 9:
                    proj.append(lambda: load_wo(0))
                    proj.append(lambda: load_wo(1))
                    proj.append(lambda: load_wo(2))
                attn, carry = attn_units(r - 1)
                # previous pair's deferred norm_b goes right after the first
                # scores unit (before this pair's first pv_unit reuses po)
                attn.insert(1, carry_box[0]) if carry_box[0] else None
                carry_box[0] = carry
                emit_round(attn, proj)

            # ---- round 10: attention heads 18/19 + O-projection ----
            o_ps = {}

            def o_chain_unit(i, j, kk, pool, ptag):
                def f():
                    if kk == 0:
                        o_ps[(i, j)] = pool.tile([P, 512], f32, tag=ptag,
                                                 name="o_ps_t")
                    nc.tensor.matmul(
                        o_ps[(i, j)][:, 0:512],
                        lhsT=wo_tiles[i][:, kk, :],
                        rhs=at_sb[:, kk, j * 512:(j + 1) * 512],
                        start=(kk == 0),
                        stop=(kk == NCT - 1),
                    )
                return f

            def o_evict_unit(i, j):
                def f():
                    yt_t = ytp.tile([P, 512], f32, tag="yt", name="yt_t")
                    nc.vector.tensor_scalar_add(
                        out=yt_t, in0=o_ps[(i, j)], scalar1=bo_sb[:, i:i + 1]
                    )
                    eng = nc.gpsimd if (i + j) % 2 == 0 else nc.sync
                    eng.dma_start(
                        out=yt_d[i][:, j * 512:(j + 1) * 512], in_=yt_t
                    )
                return f

            attn10, carry10 = attn_units(NCT - 1)
            if carry_box[0]:
                attn10.insert(1, carry_box[0])
            # chain 0's two half-chains ride along with the last attention
            # pair as tensor filler (pjp only; psp is still cycling scores
            # tiles).  They may only touch at-tiles 0..8 — tile 9 is
            # produced by this round's attention, and Tile deps are
            # emission-ordered.
            o_early = [lambda: load_wo(3), lambda: load_wo(4), lambda: load_wo(5)]
            o_early += [o_chain_unit(0, 0, kk, pjp, "pj") for kk in range(NCT - 1)]
            o_early += [o_chain_unit(0, 1, kk, pjp, "pj") for kk in range(NCT - 1)]
            emit_round(attn10, o_early, front=6)
            carry10()
            o_chain_unit(0, 0, NCT - 1, pjp, "pj")()
            o_chain_unit(0, 1, NCT - 1, pjp, "pj")()
            o_evict_unit(0, 0)()
            o_evict_unit(0, 1)()
            for i in range(1, NCT):
                if i + 5 < NCT:
                    load_wo(i + 5)
                for j in (0, 1):
                    pool, ptag = (psp, "ps") if i % 2 == 1 else (pjp, "pj")
                    for kk in range(NCT):
                        o_chain_unit(i, j, kk, pool, ptag)()
                    o_evict_unit(i, j)()

    nc.finalize()
    return nc


def kernel(**inputs):
    global last_exec_time_ns
    _install_profile_hook()
    import ml_dtypes
    from concourse.bass_utils import run_bass_kernel_spmd

    bf = ml_dtypes.bfloat16
    hs = np.asarray(inputs["hidden_states"], dtype=np.float32)
    wqt = np.ascontiguousarray(np.asarray(inputs["Wq"], np.float32).T.astype(bf))
    wkt = np.ascontiguousarray(np.asarray(inputs["Wk"], np.float32).T.astype(bf))
    wvt = np.ascontiguousarray(np.asarray(inputs["Wv"], np.float32).T.astype(bf))
    wot = np.ascontiguousarray(np.asarray(inputs["Wo"], np.float32).T.astype(bf))
    bo = np.asarray(inputs["bo"], np.float32).reshape(NCT, P).T.copy()

    if "nc" not in _cache:
        _cache["nc"] = _build()
    nc = _cache["nc"]

    in_maps = [
        {
            "xt": np.ascontiguousarray(hs[b].T.astype(bf)),
            "wq": wqt, "wk": wkt, "wv": wvt, "wo": wot, "bo": bo,
        }
        for b in range(B)
    ]
    res = run_bass_kernel_spmd(nc, in_maps, list(range(B)))
    last_exec_time_ns = res.exec_time_ns
    out = np.stack([res.results[b]["yt"].T for b in range(B)], axis=0)
    return np.ascontiguousarray(out.astype(np.float32))


# revision 19
# speedup vs baseline: 1.2241x; 1.0056x over previous
"""TRN2 Bass kernel for nn_CustomAttnProcessor (B=8, S=1024, C=1280, H=20).

Strategy (v3 — interleaved software pipeline, all-bf16 inputs):
  - Batch-parallel: one batch element per NeuronCore (8 cores), no collectives.
  - Host pre-transposes inputs (xt = X[b].T, w*t = W.T) and casts to bf16;
    kernel emits Y^T in fp32; host transposes back.
  - v1 ran the whole attention phase with the PE HAM clock-gate stuck at
    K=4/8 (1.2 GHz): scores->exp->PV left too many PE idle gaps.  v2/v3
    interleave the Q/K/V projections of c-tile ct+1 with the attention of
    heads 2ct/2ct+1 at matmul granularity, keeping the PE densely busy
    (warm, 2.4 GHz) and hiding ~180us of serial ScalarE exp behind tensor
    work.  The O-projection runs as a dense tail, with its first chain
    pulled into the last attention round as filler (at-tiles 0..8 only —
    tile 9 is produced by that round, and Tile deps are emission-ordered).
  - Head-pair scores are emitted back-to-back so bass's auto tile_position
    ((0,0) for even heads, (64,0) for odd heads, tile_size (64,128)) lets
    the PE overlap the two 64-contraction matmuls (2x row tiling).
  - q is processed in halves of 512 so the PV accumulators for a head pair
    fit in 2 PSUM banks ([65, 512+512]); scores pair tiles are [128, 1024]
    (2 banks, one exp ACTIVATE of N=1024 covers both heads).
  - All matmul operands bf16 (inputs rounded on host; rel err ~5e-3 vs the
    2e-2 gate), accumulation fp32 in PSUM.  bf16 halves HBM traffic and
    SBUF footprint vs fp32r and enables FWL weight loads.
  - PV uses the ones-augmented-V trick: stationary [128, 65] whose 65th
    column accumulates the softmax denominator for free.
"""

import sys
import types

import numpy as np

B, S, C, H = 8, 1024, 1280, 20
D = C // H              # 64 head dim
P = 128
NCT = C // P            # 10 c-tiles
NST = S // P            # 8 s-tiles
SCALE = float(D) ** -0.5

# V-projection chunks: (col_offset, width) and the rounds their m-chains run
V_CHUNKS = [(0, 256), (256, 512), (768, 512)]

_cache = {}

last_exec_time_ns = None


def _install_profile_hook():
    """antenv.axon_hooks is absent in this container; reconstruct it from
    trn_agent_boot so BASS_TRACE=1 profiling works."""
    if "antenv.axon_hooks" in sys.modules:
        return
    mod = types.ModuleType("antenv.axon_hooks")
    state = {"hook": None}
    mod.set_axon_ntff_profile_hook = lambda h: state.update(hook=h)
    mod.get_axon_ntff_profile_hook = lambda: state["hook"]
    sys.modules["antenv.axon_hooks"] = mod
    try:
        from trn_agent_boot.trn_boot import _ntff_profile_via_ctypes

        hook = _ntff_profile_via_ctypes("/opt/axon/libaxon_pjrt.so")
        if hook is not None:
            mod.set_axon_ntff_profile_hook(hook)
    except Exception:
        pass


def _build():
    import concourse.mybir as mybir
    from concourse import bacc
    from concourse.tile import TileContext

    f32 = mybir.dt.float32
    bf16 = mybir.dt.bfloat16
    EXP = mybir.ActivationFunctionType.Exp

    nc = bacc.Bacc()
    xt = nc.declare_dram_parameter("xt", [C, S], bf16, isOutput=False)
    # weights arrive pre-tiled from the host so every DMA is a long
    # contiguous read (the naive "(t p) o" gather has 256B runs on bf16
    # and measured ~5x below DMA spec):
    #   wq/wk/wo: [ct, p, t, o];  wv: [p, t, c_out]
    wq = nc.declare_dram_parameter("wq", [NCT, P, NCT, P], bf16, isOutput=False)
    wk = nc.declare_dram_parameter("wk", [NCT, P, NCT, P], bf16, isOutput=False)
    wv = nc.declare_dram_parameter("wv", [P, NCT, C], bf16, isOutput=False)
    wo = nc.declare_dram_parameter("wo", [NCT, P, NCT, P], bf16, isOutput=False)
    bo = nc.declare_dram_parameter("bo", [P, NCT], f32, isOutput=False)
    yt = nc.declare_dram_parameter("yt", [C, S], f32, isOutput=True)

    with TileContext(nc) as tc:
        with (
            tc.tile_pool(name="big", bufs=1) as big,
            tc.tile_pool(name="wqk", bufs=2) as wqkp,
            tc.tile_pool(name="wvp", bufs=2) as wvp,
            tc.tile_pool(name="wop", bufs=6) as wop,
            tc.tile_pool(name="ptp", bufs=10) as ptp,
            tc.tile_pool(name="rbp", bufs=2) as rbp,
            tc.tile_pool(name="rcpp", bufs=1) as rcpp,
            tc.tile_pool(name="ytp", bufs=4) as ytp,
            tc.tile_pool(name="smt", bufs=1) as smt,
            tc.tile_pool(name="psp", bufs=2, space="PSUM") as psp,
            tc.tile_pool(name="pop", bufs=1, space="PSUM") as pop,
            tc.tile_pool(name="pjp", bufs=2, space="PSUM") as pjp,
        ):
            # ---- persistent tiles ----
            xt_sb = big.tile([P, NCT, S], bf16, tag="xt")
            qt_sb = big.tile([P, NCT, S], bf16, tag="qt")
            kt_sb = big.tile([P, NCT, S], bf16, tag="kt")
            v_sb = big.tile([P, NST, H * 65], bf16, tag="v")
            at_sb = big.tile([P, NCT, S], bf16, tag="at")
            bo_sb = smt.tile([P, NCT], f32, tag="bo")
            ones_sb = smt.tile([P, H], bf16, tag="ones")

            xt_d = xt.rearrange("(t p) s -> t p s", p=P)
            yt_d = yt.rearrange("(t p) s -> t p s", p=P)

            # ---- weight staging ----
            wqk_tiles = {}

            def load_wqk(ct):
                t = wqkp.tile([P, NCT, 2 * P], bf16, tag="wqk", name="wqk_t")
                nc.sync.dma_start(out=t[:, :, 0:P], in_=wq[ct])
                nc.sync.dma_start(out=t[:, :, P:2 * P], in_=wk[ct])
                wqk_tiles[ct] = t

            wv_tiles = {}

            def load_wv(ch):
                off, width = V_CHUNKS[ch]
                t = wvp.tile([P, NCT, width], bf16, tag="wv", name="wv_t")
                nc.sync.dma_start(out=t, in_=wv[:, :, off:off + width])
                wv_tiles[ch] = t

            wo_tiles = {}

            def load_wo(i):
                t = wop.tile([P, NCT, P], bf16, tag="wo", name="wo_t")
                nc.scalar.dma_start(out=t, in_=wo[i])
                wo_tiles[i] = t

            # ---- prologue DMAs, in consumption order, two queues ----
            # Per-DMA queue-issue overhead is ~0.9us, so split across the
            # sync and scalar (Activation) DGE queues and order by first use:
            # xt0/xt1, then the Q/K weight halves, then remaining xt chunks.
            t0 = wqkp.tile([P, NCT, 2 * P], bf16, tag="wqk", name="wqk_t")
            wqk_tiles[0] = t0
            nc.sync.dma_start(out=xt_sb[:, 0, :], in_=xt_d[0])
            nc.scalar.dma_start(out=xt_sb[:, 1, :], in_=xt_d[1])
            nc.sync.dma_start(out=t0[:, :, 0:P], in_=wq[0])
            nc.scalar.dma_start(out=t0[:, :, P:2 * P], in_=wk[0])
            for k in range(2, NCT):
                eng = (nc.sync, nc.scalar, nc.gpsimd)[k % 3]
                eng.dma_start(out=xt_sb[:, k, :], in_=xt_d[k])
            nc.gpsimd.dma_start(out=bo_sb, in_=bo[:, :])
            nc.vector.memset(ones_sb, 1.0)
            # wv0 on the scalar queue so it lands before the first V chain
            # needs it (the sync queue is busy with xt chunks + wqk1)
            t = wvp.tile([P, NCT, V_CHUNKS[0][1]], bf16, tag="wv", name="wv_t")
            nc.scalar.dma_start(out=t, in_=wv[:, :, 0:V_CHUNKS[0][1]])
            wv_tiles[0] = t
            load_wqk(1)
            for m in range(NST):
                nc.vector.tensor_copy(
                    v_sb[:, m, :].rearrange("p (h e) -> p h e", e=65)[:, :, 64:65],
                    ones_sb[:, :, None],
                )

            # ---- projection work units ----
            # Chains run in single-bank [128, 512] half-chains so the pjp
            # pool (2 one-bank slots) decouples consecutive chains: the
            # next chain's k=0 WAR lands ~10 units after the previous
            # half's eviction instead of immediately behind it on the DVE.
            def qk_chain_units(ct, which):
                wt = wqk_tiles[ct]
                off = 0 if which == "q" else P
                dst = qt_sb if which == "q" else kt_sb
                box = {}

                def mk(j, k):
                    def f():
                        if k == 0:
                            box[j] = pjp.tile([P, 512], f32, tag="pj",
                                              name="pj_ps")
                        nc.tensor.matmul(
                            box[j][:, 0:512],
                            lhsT=wt[:, k, off:off + P],
                            rhs=xt_sb[:, k, j * 512:(j + 1) * 512],
                            start=(k == 0),
                            stop=(k == NCT - 1),
                        )
                    return f

                def ev(j):
                    def f():
                        nc.vector.tensor_copy(
                            dst[:, ct, j * 512:(j + 1) * 512], box[j]
                        )
                    return f

                units = []
                for j in (0, 1):
                    units += [mk(j, k) for k in range(NCT)]
                    units.append(ev(j))
                return units

            def v_chain_unit(ch, m):
                off, width = V_CHUNKS[ch]
                h0 = off // D
                nh = width // D

                def f():
                    pv = pjp.tile([P, 512], f32, tag="pj", name="pj_pv")
                    wt = wv_tiles[ch]
                    for k in range(NCT):
                        nc.tensor.matmul(
                            pv[:, 0:width],
                            lhsT=xt_sb[:, k, m * P:(m + 1) * P],
                            rhs=wt[:, k, :],
                            start=(k == 0),
                            stop=(k == NCT - 1),
                        )
                    dst = v_sb[:, m, :].rearrange("p (h e) -> p h e", e=65)[
                        :, h0:h0 + nh, 0:64
                    ]
                    nc.vector.tensor_copy(
                        dst, pv[:, 0:width].rearrange("p (h e) -> p h e", e=64)
                    )
                return f

            # ---- attention work units for head pair (2ct, 2ct+1) ----
            def attn_units(cth):
                hA, hB = 2 * cth, 2 * cth + 1
                pts = {}
                po_box = {}

                def sc_unit(qh, kt):
                    def f():
                        ps = psp.tile([P, S], f32, tag="ps", name="ps_sc")
                        nc.tensor.matmul(
                            ps[:, 0:512],
                            lhsT=kt_sb[0:64, cth, kt * P:(kt + 1) * P],
                            rhs=qt_sb[0:64, cth, qh * 512:(qh + 1) * 512],
                            start=True, stop=True,
                        )
                        nc.tensor.matmul(
                            ps[:, 512:1024],
                            lhsT=kt_sb[64:128, cth, kt * P:(kt + 1) * P],
                            rhs=qt_sb[64:128, cth, qh * 512:(qh + 1) * 512],
                            start=True, stop=True,
                        )
                        pt = ptp.tile([P, S], bf16, tag="pt", name="pt_exp")
                        nc.scalar.activation(out=pt, in_=ps, func=EXP, scale=SCALE)
                        pts[(qh, kt)] = pt
                    return f

                def pv_unit(qh, kt):
                    def f():
                        if kt == 0:
                            po_box[qh] = pop.tile([65, S], f32, tag="po",
                                                  name="po_acc")
                        po_t = po_box[qh]
                        pt = pts.pop((qh, kt))
                        nc.tensor.matmul(
                            po_t[:, 0:512],
                            lhsT=v_sb[:, kt, 65 * hA:65 * hA + 65],
                            rhs=pt[:, 0:512],
                            start=(kt == 0), stop=(kt == NST - 1),
                        )
                        nc.tensor.matmul(
                            po_t[:, 512:1024],
                            lhsT=v_sb[:, kt, 65 * hB:65 * hB + 65],
                            rhs=pt[:, 512:1024],
                            start=(kt == 0), stop=(kt == NST - 1),
                        )
                    return f

                rb_box = {}

                def norm_a_unit(qh):
                    def f():
                        po_t = po_box[qh]
                        den = rcpp.tile([1, S], f32, tag="den", name="den_t")
                        nc.vector.tensor_copy(den, po_t[64:65, :])
                        rcp = rcpp.tile([1, S], f32, tag="rcp", name="rcp_t")
                        nc.vector.reciprocal_approx_fast(out=rcp, in_=den)
                        rb = rbp.tile([64, S], f32, tag="rb", name="rb_t")
                        nc.gpsimd.partition_broadcast(rb, rcp)
                        rb_box[qh] = rb
                    return f

                def norm_b_unit(qh):
                    # must be emitted before the next pv_unit(kt==0) that
                    # reuses the single po slot (Tile deps are emission-
                    # ordered); splitting lets independent DVE work overlap
                    # the gpsimd broadcast latency.
                    def f():
                        po_t = po_box[qh]
                        rb = rb_box[qh]
                        nc.vector.tensor_mul(
                            at_sb[0:64, cth, qh * 512:(qh + 1) * 512],
                            po_t[0:64, 0:512], rb[:, 0:512],
                        )
                        nc.vector.tensor_mul(
                            at_sb[64:128, cth, qh * 512:(qh + 1) * 512],
                            po_t[0:64, 512:1024], rb[:, 512:1024],
                        )
                    return f

                def sc2_unit(qh, kt2):
                    # two score-pair steps back-to-back: keeps the PE in
                    # 64x128 tiling mode for 4 MMs (mode switches cost
                    # ~100ns of drain each)
                    a, b = sc_unit(qh, kt2), sc_unit(qh, kt2 + 1)

                    def f():
                        a()
                        b()
                    return f

                def pv2_unit(qh, kt2):
                    a, b = pv_unit(qh, kt2), pv_unit(qh, kt2 + 1)

                    def f():
                        a()
                        b()
                    return f

                units = []
                for qh in (0, 1):
                    # all scores first (PV lag 6): the first pv_unit of a
                    # pass reuses the single po slot, whose release waits on
                    # the previous pass's norm chain (gpsimd broadcast +
                    # DVE muls, ~3.4us); deep lag hides that entirely.
                    for kt2 in range(0, NST, 2):
                        units.append(sc2_unit(qh, kt2))
                        if qh == 1 and kt2 == 6:
                            units.append(norm_b_unit(0))
                        if kt2 >= 6:
                            units.append(pv2_unit(qh, kt2 - 6))
                    units.append(pv2_unit(qh, NST - 6))
                    units.append(pv2_unit(qh, NST - 4))
                    units.append(pv2_unit(qh, NST - 2))
                    units.append(norm_a_unit(qh))
                # norm_b(1) is returned as carry: the caller emits it early
                # in the NEXT round, before that pair's first pv_unit.
                return units, norm_b_unit(1)

            def emit_round(attn, proj, front=2):
                if not attn:
                    for u in proj:
                        u()
                    return
                j = 0
                # front-load proj units so the tensor queue is fed while
                # the first scores' inputs settle
                while j < min(front, len(proj)):
                    proj[j]()
                    j += 1
                for i, u in enumerate(attn):
                    u()
                    target = ((i + 1) * len(proj)) // len(attn)
                    while j < target:
                        proj[j]()
                        j += 1
                while j < len(proj):
                    proj[j]()
                    j += 1

            carry_box = [None]

            # ---- round 0 (prologue): Q0, K0, V chunk 0 ----
            proj0 = qk_chain_units(0, "q") + qk_chain_units(0, "k")
            proj0 += [v_chain_unit(0, m) for m in range(NST)]
            emit_round(None, proj0)
            load_wv(1)

            # ---- rounds 1..9 ----
            # V chunk 1 (cols 256:768) m-chains in rounds 1-2;
            # V chunk 2 (cols 768:1280) m-chains in rounds 3-6.
            for r in range(1, NCT):
                proj = []
                if r + 1 < NCT:
                    proj.append(lambda ct=r + 1: load_wqk(ct))
                proj += qk_chain_units(r, "q") + qk_chain_units(r, "k")
                if r in (1, 2):
                    ms = range(0, 4) if r == 1 else range(4, NST)
                    proj += [v_chain_unit(1, m) for m in ms]
                    if r == 2:
                        proj.append(lambda: load_wv(2))
                elif 3 <= r <= 6:
                    proj += [v_chain_unit(2, m) for m in (2 * (r - 3),
                                                          2 * (r - 3) + 1)]
                if r == 9:
                    proj.append(lambda: load_wo(0))
                    proj.append(lambda: load_wo(1))
                    proj.append(lambda: load_wo(2))
                attn, carry = attn_units(r - 1)
                # previous pair's deferred norm_b goes right after the first
                # scores unit (before this pair's first pv_unit reuses po)
                attn.insert(1, carry_box[0]) if carry_box[0] else None
                carry_box[0] = carry
                emit_round(attn, proj)

            # ---- round 10: attention heads 18/19 + O-projection ----
            o_ps = {}

            def o_chain_unit(i, j, kk, pool, ptag):
                def f():
                    if kk == 0:
                        o_ps[(i, j)] = pool.tile([P, 512], f32, tag=ptag,
                                                 name="o_ps_t")
                    nc.tensor.matmul(
                        o_ps[(i, j)][:, 0:512],
                        lhsT=wo_tiles[i][:, kk, :],
                        rhs=at_sb[:, kk, j * 512:(j + 1) * 512],
                        start=(kk == 0),
                        stop=(kk == NCT - 1),
                    )
                return f

            def o_evict_unit(i, j):
                def f():
                    yt_t = ytp.tile([P, 512], f32, tag="yt", name="yt_t")
                    nc.vector.tensor_scalar_add(
                        out=yt_t, in0=o_ps[(i, j)], scalar1=bo_sb[:, i:i + 1]
                    )
                    eng = nc.gpsimd if (i + j) % 2 == 0 else nc.sync
                    eng.dma_start(
                        out=yt_d[i][:, j * 512:(j + 1) * 512], in_=yt_t
                    )
                return f

            attn10, carry10 = attn_units(NCT - 1)
            if carry_box[0]:
                attn10.insert(1, carry_box[0])
            # chain 0's two half-chains ride along with the last attention
            # pair as tensor filler (pjp only; psp is still cycling scores
            # tiles).  They may only touch at-tiles 0..8 — tile 9 is
            # produced by this round's attention, and Tile deps are
            # emission-ordered.
            o_early = [lambda: load_wo(3), lambda: load_wo(4), lambda: load_wo(5)]
            o_early += [o_chain_unit(0, 0, kk, pjp, "pj") for kk in range(NCT - 1)]
            o_early += [o_chain_unit(0, 1, kk, pjp, "pj") for kk in range(NCT - 1)]
            emit_round(attn10, o_early, front=6)
            carry10()
            o_chain_unit(0, 0, NCT - 1, pjp, "pj")()
            o_chain_unit(0, 1, NCT - 1, pjp, "pj")()
            o_evict_unit(0, 0)()
            o_evict_unit(0, 1)()
            for i in range(1, NCT):
                if i + 5 < NCT:
                    load_wo(i + 5)
                for j in (0, 1):
                    pool, ptag = (psp, "ps") if i % 2 == 1 else (pjp, "pj")
                    for kk in range(NCT):
                        o_chain_unit(i, j, kk, pool, ptag)()
                    o_evict_unit(i, j)()

    nc.finalize()
    return nc


def kernel(**inputs):
    global last_exec_time_ns
    _install_profile_hook()
    import ml_dtypes
    from concourse.bass_utils import run_bass_kernel_spmd

    bf = ml_dtypes.bfloat16

    def tile_qko(w):
        # W.T is [c_in, c_out]; emit [ct, p, t, o] with c_in = t*P+p,
        # c_out = ct*P+o so each per-ct tile DMA is fully contiguous
        a = np.asarray(w, np.float32).T.astype(bf).reshape(NCT, P, NCT, P)
        return np.ascontiguousarray(a.transpose(2, 1, 0, 3))

    def tile_v(w):
        a = np.asarray(w, np.float32).T.astype(bf).reshape(NCT, P, C)
        return np.ascontiguousarray(a.transpose(1, 0, 2))  # [p, t, c_out]

    hs = np.asarray(inputs["hidden_states"], dtype=np.float32)
    wqt = tile_qko(inputs["Wq"])
    wkt = tile_qko(inputs["Wk"])
    wvt = tile_v(inputs["Wv"])
    wot = tile_qko(inputs["Wo"])
    bo = np.asarray(inputs["bo"], np.float32).reshape(NCT, P).T.copy()

    if "nc" not in _cache:
        _cache["nc"] = _build()
    nc = _cache["nc"]

    in_maps = [
        {
            "xt": np.ascontiguousarray(hs[b].T.astype(bf)),
            "wq": wqt, "wk": wkt, "wv": wvt, "wo": wot, "bo": bo,
        }
        for b in range(B)
    ]
    res = run_bass_kernel_spmd(nc, in_maps, list(range(B)))
    last_exec_time_ns = res.exec_time_ns
    out = np.stack([res.results[b]["yt"].T for b in range(B)], axis=0)
    return np.ascontiguousarray(out.astype(np.float32))
